# revision 9
# baseline (speedup 1.0000x reference)
"""Correlation-volume kernel for trn2 (8 NeuronCores, batch-parallel).

out[n, (i,j), h, w] = sum_z imgA[n,z,h,w] * imgB[n,z,h+(j-4),w+(i-4)]
(zero padding outside the image; equivalent to the bilinear reference
since all offsets are integral).

v6 device strategy (per core, one batch element):
  - inputs cast to fp16 on host; B zero-padded to 168x168 on host; A
    re-laid-out block-major on host (walrus: matmul weights AP must
    have a single free dim).
  - 200 stationary blocks of 16x8=128 A-pixels. Each block is computed
    with FOUR col-tiled matmuls (tile i = pixel rows [4i,4i+4), 32
    stationary cols at PE array col-group i). Tile i's moving operand
    is the B window rows [16bh+4i, +12) x 16 cols -- the per-tile row
    offset absorbs most of the band shear, so PSUM holds only 192
    cols/pixel. The 4 tile matmuls stream concurrently through
    different PE column groups (measured ~185ns/block cadence).
  - PSUM: one [128, 4096] f32 tensor = all 8 banks. Block b lands at
    col offset (b%16)*256 (192 used; 256-aligned, no bank crossing).
    16 blocks in flight.
  - copies PSUM->SBUF (cast fp16) in 8-block chunks: one DVE/ACT op
    per chunk (alternating engines), 4-D AP [p, u:12, sl:8, v:16].
    Measured: DVE ~1.76us, ACT ~1.54us per chunk -> copies pace at
    ~9.7 blocks/us, never the bottleneck.
  - chunks staged in groups of 96 blocks (12 chunks; last group = 8).
  - spills run on the GPSIMD SWDGE ring, CONCURRENT with the input
    loads on the SP HWDGE ring (one ring serializes: loads 32us THEN
    spills; and ACT-issued spills backpressure-stall the ACT copy
    stream). Spill granularity: one DMA per (group, pixel-row PAIR):
    rows 2q,2q+1 share window rows u' in [s2, s2+10), s2=2*(q%2), at
    the cost of spilling 10 instead of 9 u-rows (+11% bytes, 8.19MB).
    That makes each DMA [16 partitions x 30.7KB contiguous] = 492KB --
    big per-DMA footprint is what sustains HBM write bandwidth (the
    ring keeps only ~3.5 DMAs in flight; 8-partition 14KB-run DMAs
    measured only ~105GB/s aggregate). The 9-of-16 col band pick (and
    the u''/row-parity unshear) happens on host during the unshard.

Sync notes: distinct DMAs complete OUT OF ORDER across queues, so every
DMA that gates compute gets its own semaphore (per input strip-pair,
per staging slot). Engine-side increments (matmul, copies) are in-order
per engine. Matmuls of one block complete in pc order, so only the
last tile's matmul incs s_mm.
"""

import numpy as np
from numpy.lib.stride_tricks import as_strided

import concourse.bass as bass
import concourse.mybir as mybir
from concourse.bass_utils import run_bass_kernel_spmd

F16 = mybir.dt.float16
F32 = mybir.dt.float32

Z = 128
H = W = 160
PAD = 4
R = 9                      # displacements per axis
BH, BW = 16, 8             # stationary block (BH*BW == 128)
NBH, NBW = H // BH, W // BW    # 10 x 20
NB = NBH * NBW             # 200 blocks
TIL = 4                    # col tiles per block (32 partitions each)
TROWS = BH // TIL          # pixel rows per tile = 4
TMH = TROWS + 2 * PAD      # 12 moving rows per tile
MW = BW + 2 * PAD          # 16 moving cols
TFD = TMH * MW             # 192 psum cols per block
POFF = 256                 # psum col stride per block (192 used)
SLAB = R * MW              # useful band per pixel: 9*16 = 144
HP, WP = H + 2 * PAD, W + 2 * PAD     # 168 x 168 padded B
CHK = 8                    # blocks per copy chunk
NCH = NB // CHK            # 25 chunks
CPG = 12                   # chunks per staging group (last group: 13)
NGRP = 2                   # staging groups: 96,104 blocks
GSIZE = [96, 104]
GBASE = [0, 96]
ROWCH = [gs * MW for gs in GSIZE]      # stage els per window row u'
ROWMAX = max(ROWCH)        # 1664: dram row stride per partition / PR
STGW = [TMH * gs * MW for gs in GSIZE]
NSTG = 2
NPAIR = BH // 2            # 8 row-pair spill DMAs per group
PR = 10                    # spilled u'' rows per pair (9 + 1 overspill)
STRIP = 32                 # input load strip (rows)
NWARM = 32                 # PE warmup matmuls (HAM un-throttle)

NP_F16 = np.float16

NBS = (HP + STRIP - 1) // STRIP   # 6 B strips (last is 8 rows)
NAS = H // STRIP                  # 5 A strips


def _grp_of(cc):
    return min(cc // CPG, NGRP - 1)


def _strips_needed(bh):
    """(jb, ja): last B strip and last A strip block-row bh depends on."""
    jb = (BH * bh + BH + 2 * PAD - 1) // STRIP
    ja = (BH * bh + BH - 1) // STRIP
    return jb, ja


def build_nc():
    nc = bass.Bass()
    a = nc.declare_dram_parameter("a", [Z, H * W], F16, isOutput=False)
    bp = nc.declare_dram_parameter("bp", [Z, HP * WP], F16, isOutput=False)
    # spill: one row per (group, row-pair) DMA;
    # row = [r2: 2][c: 8][u'': 10][sl: gsize][v: 16]
    g = nc.declare_dram_parameter(
        "g", [NGRP * NPAIR, 16 * PR * ROWMAX], F16, isOutput=True
    )

    # one sem per strip index j: B_j incs +16, A_j (j<NAS) incs +16.
    s_ld = [nc.alloc_semaphore(f"s_ld{j}") for j in range(NBS)]
    s_sp = [nc.alloc_semaphore(f"s_sp{i}") for i in range(NSTG)]

    with (
        nc.sbuf_tensor([Z, H * W], F16) as a_sb,
        nc.sbuf_tensor([Z, HP * WP], F16) as b_sb,
        nc.sbuf_tensor([Z, STGW[0]], F16) as stage0,
        nc.sbuf_tensor([Z, STGW[1]], F16) as stage1,
        nc.psum_tensor([Z, 4096], F32) as ps,
        nc.semaphore("s_mm") as s_mm,
        nc.semaphore("s_cpv") as s_cpv,
        nc.semaphore("s_cpa") as s_cpa,
        nc.Block(no_gpsimd_drain=True) as block,
    ):
        stage = [stage0, stage1]
        b3 = b_sb[:].rearrange("p (h w) -> p h w", h=HP)
        b3d = bp[:].rearrange("p (h w) -> p h w", h=HP)
        # spill dst row (gi, q): [pc: 16][x] — partition dim outermost
        # on the src side (BIR verifier requirement).
        g4 = g[:].rearrange("d (c x) -> d c x", c=16)

        # copy views: src psum chunk [p, u:12, sl:8, v:16] (sl stride
        # POFF), dst stage section [p, u:12, sl-slice, v:16].
        ps_r = ps[:].rearrange("p (b x) -> p b x", x=POFF)
        src_chunk = [
            ps_r[:, h * CHK:(h + 1) * CHK, 0:TFD].rearrange(
                "p b (u v) -> p u b v", v=MW
            )
            for h in range(2)
        ]
        stage_g = [
            s[:].rearrange("p (u b v) -> p u b v", b=GSIZE[i], v=MW)
            for i, s in enumerate(stage)
        ]

        def _copy_dst(cc):
            gi = _grp_of(cc)
            cs = cc - gi * CPG
            return stage_g[gi][:, :, CHK * cs:CHK * (cs + 1), :]

        def _copy_waits(eng, cc):
            # NGRP == NSTG: stage slots are never reused.
            eng.wait_ge(s_mm, CHK * (cc + 1))

        @block.scalar
        def _(scalar):
            # odd chunks' copies
            for cc in range(1, NCH, 2):
                _copy_waits(scalar, cc)
                nc.scalar.copy(
                    _copy_dst(cc), src_chunk[cc % 2]
                ).then_inc(s_cpa, 1)

        @block.vector
        def _(vector):
            # even chunks' copies
            for cc in range(0, NCH, 2):
                _copy_waits(vector, cc)
                nc.vector.tensor_copy(
                    _copy_dst(cc), src_chunk[cc % 2]
                ).then_inc(s_cpv, 1)

        @block.tensor
        def _(tensor):
            # HAM warmup: dense dummy matmuls on scratch data so the PE
            # clock is at 8/8 before the real stream begins. Results
            # land in bank 0, overwritten by block 0 (start=True).
            for _ in range(NWARM):
                nc.tensor.matmul(
                    ps[:, 0:TFD],
                    stage0[:, 0:128],
                    stage1[:, 0:TFD],
                    start=True,
                    stop=True,
                )
            waited = set()
            for b in range(NB):
                bh, bw = divmod(b, NBW)
                if bw == 0:
                    jb, ja = _strips_needed(bh)
                    for j in range(jb + 1):
                        if j not in waited:
                            need = 32 if j < NAS else 16
                            tensor.wait_ge(s_ld[j], need)
                            waited.add(j)
                cc = b // CHK
                if b % CHK == 0 and cc >= 2:
                    # psum half (cc%2) free once copy of chunk cc-2 done
                    tensor.wait_ge(s_cpv if cc % 2 == 0 else s_cpa, cc // 2)
                off = (b % 16) * POFF
                h0, w0 = bh * BH, bw * BW
                for i in range(TIL):
                    mm = nc.tensor.matmul(
                        ps[32 * i:32 * (i + 1), off:off + TFD],
                        a_sb[:, b * 128 + 32 * i:b * 128 + 32 * (i + 1)],
                        b3[:, h0 + TROWS * i:h0 + TROWS * i + TMH,
                           w0:w0 + MW],
                        start=True,
                        stop=True,
                        # bass_types.rs restricts base_partition() to
                        # {0,32,64}; the 4th col tile must be explicit.
                        tile_position=(0, 32 * i),
                    )
                    if i == TIL - 1:
                        mm.then_inc(s_mm, 1)

        @block.gpsimd
        def _(gpsimd):
            # row-pair slab spills on the SWDGE ring — concurrent with
            # the input loads on the SP HWDGE ring.
            for gi in range(NGRP):
                end = CPG * (gi + 1) if gi < NGRP - 1 else NCH
                gpsimd.wait_ge(s_cpv, (end + 1) // 2)
                gpsimd.wait_ge(s_cpa, end // 2)
                rowch = ROWCH[gi]
                for q in range(NPAIR):
                    s2 = 2 * (q % 2)
                    src = stage[gi][
                        16 * q:16 * (q + 1),
                        rowch * s2:rowch * (s2 + PR),
                    ]
                    gpsimd.dma_start(
                        out=g4[gi * NPAIR + q][:, 0:PR * rowch], in_=src
                    ).then_inc(s_sp[gi % NSTG], 16)

        @block.sync
        def _(sync):
            # input strip loads (no waits -> issue immediately, FIFO).
            # "a" is block-major on host: a strip of STRIP image rows
            # is a whole number of block rows = contiguous columns.
            for j in range(NBS):
                r0, r1 = j * STRIP, min((j + 1) * STRIP, HP)
                sync.dma_start(
                    out=b3[:, r0:r1, :], in_=b3d[:, r0:r1, :]
                ).then_inc(s_ld[j], 16)
                if j < NAS:
                    c0, c1 = j * STRIP * W, (j + 1) * STRIP * W
                    sync.dma_start(
                        out=a_sb[:, c0:c1], in_=a[:, c0:c1]
                    ).then_inc(s_ld[j], 16)
            for i in range(NSTG):
                ngrp = len([x for x in range(NGRP) if x % NSTG == i])
                sync.wait_ge(s_sp[i], 16 * NPAIR * ngrp)

    return nc


def prep_core(An, Bn):
    """An, Bn: [Z,H,W] float32 -> per-core input map (fp16, B padded).

    "a" is laid out block-major: [z, bh, bw, h_l, w_l] so each stationary
    block's 128 pixels are contiguous (walrus: weights AP must be 1-D free).
    Tile i of a block = rows [4i, 4i+4) = els [32i, 32i+32) of the block.
    """
    a = (
        An.reshape(Z, NBH, BH, NBW, BW)
        .transpose(0, 1, 3, 2, 4)
        .reshape(Z, H * W)
        .astype(NP_F16)
    )
    bpad = np.zeros((Z, HP, WP), NP_F16)
    bpad[:, PAD:PAD + H, PAD:PAD + W] = Bn
    return {"a": np.ascontiguousarray(a), "bp": bpad.reshape(Z, HP * WP)}


def extract_core(gres):
    """gres: [NGRP*NPAIR, 16*PR*ROWMAX] fp16 spill -> [81,H,W] float32.

    Spill row (gi, q) holds [r2: 2][c: 8][u'': PR][sl: gsize][v: 16]
    for pixel (2q+r2, c) of block GBASE[gi]+sl: dy4 = u'' - r2 and the
    value for (dy4, dx4) sits at v = c + dx4.
    """
    gres = np.ascontiguousarray(gres)
    big = np.empty((NB, BH, 8, R, MW), np.float16)
    for gi in range(NGRP):
        gs = GSIZE[gi]
        rowch = gs * MW
        for q in range(NPAIR):
            # device writes partition c at flat offset c*PR*ROWMAX
            # regardless of group size; only the first PR*rowch els of
            # each partition run are meaningful.
            row = gres[gi * NPAIR + q].reshape(16, PR * ROWMAX)[:, :PR * rowch]
            S = row.reshape(2, 8, PR, gs, MW)   # [r2, c, u'', sl, v]
            for r2 in range(2):
                # [c, dy4, sl, v] -> [sl, c, dy4, v]
                big[GBASE[gi]:GBASE[gi] + gs, 2 * q + r2] = (
                    S[r2, :, r2:r2 + R].transpose(2, 0, 1, 3)
                )
    T = big.reshape(NBH, NBW, BH, 8, SLAB)
    st = T.strides
    out = np.empty((R * R, H, W), np.float32)
    for dx4 in range(R):
        for dy4 in range(R):
            k = dx4 * R + dy4
            base = T[:, :, :, :, MW * dy4 + dx4:]
            V = as_strided(
                base,
                shape=(NBH, NBW, BH, 8),
                strides=(st[0], st[1], st[2], st[3] + st[4]),
            )
            out[k] = (
                V.transpose(0, 2, 1, 3).astype(np.float32).reshape(H, W)
            )
    return out


_NC_CACHE = {}


def get_nc():
    if "nc" not in _NC_CACHE:
        _NC_CACHE["nc"] = build_nc()
    return _NC_CACHE["nc"]


def kernel(imgA, imgB):
    imgA = np.asarray(imgA)
    imgB = np.asarray(imgB)
    N = imgA.shape[0]
    in_maps = [prep_core(imgA[n], imgB[n]) for n in range(N)]
    res = run_bass_kernel_spmd(get_nc(), in_maps, list(range(N)))
    return np.stack([extract_core(res.results[n]["g"]) for n in range(N)])
